# revision 15
# baseline (speedup 1.0000x reference)
"""Trainium2 Bass kernel for DiffMultiHeadedAttention (differential attention).

Model (per reference):
    q = x @ Wq.T + bq; k = ef @ Wk.T + bk; v = ef @ Wv.T + bv
    lambda_full = exp(sum(lq1*lk1)) - exp(sum(lq2*lk2)) + 0.8
    att  = softmax(causal_mask(q_hh @ k_hh.T / sqrt(32)))   per 32 half-heads
    out_h = (att[2h] - lambda_full * att[2h+1]) @ v_h       per 16 heads
B=4, T=N=1024, H=16 heads of 64, 2H=32 half-heads of 32.

Sharding over 8 cores: core c = (batch b = c//2, head-group hg = c%2).
Each core owns one batch element and 8 full heads (16 half-heads) and
computes out^T [512, 1024]; the host transposes and reassembles.

v2 design (vs the v1 two-phase kernel):
  - All big DMA loads issue first (const setup overlaps the transfers).
  - lambda_full is computed on the HOST; the device gets a [1,8] fp16
    row of -1/lambda that becomes column 65 of the augmented v tile, so
    the AV matmul emits row 64 = sum(E_pos) and row 65 = -sum(E_neg)/l
    and a single fast-reciprocal yields both 1/s_pos and -l/s_neg.
  - Software-pipelined attention: the AV matmuls + combine of iteration
    i-1 are interleaved (as PE filler) between the QK pairs of
    iteration i, so the tensor engine never stalls on the scalar
    engine's exp; q/k projection matmuls for later head groups are
    ALSO drip-fed into the attention stream as additional filler.
  - Combine = 2 fast reciprocals + one broadcast DMA (partition-stride
    0) + 2 multiplies + 1 add + one output DMA; no gpsimd broadcast,
    no SWDGE accumulating DMA, PSUM read directly (no P65 copy).
"""

import math
from collections import deque

import numpy as np

B, T, N, HIDDEN = 4, 1024, 1024, 1024
H, HEAD, HALF = 16, 64, 32
O = 512            # per-core hidden slice (8 heads * 64)
HPC = 8            # heads per core
LAMBDA_INIT = 0.8
SCALE = 1.0 / math.sqrt(HALF)
P = 128
IC = HIDDEN // P   # 8 contraction chunks
OC = O // P        # 4 output chunks of the projections
NT = N // P        # 8 n-tiles (keys)
NCORES = 8

_STATE = {}


def _build_nc():
    from contextlib import ExitStack

    import concourse.bacc as bacc
    import concourse.mybir as mybir
    import concourse.tile as tile
    from concourse.ap import AP as BAP
    from concourse.bass import ts

    f32 = mybir.dt.float32
    f16 = mybir.dt.float16
    AF = mybir.ActivationFunctionType
    ALU = mybir.AluOpType

    nc = bacc.Bacc("TRN2", target_bir_lowering=False, debug=False)

    xt_d = nc.dram_tensor("xt", [HIDDEN, T], f16, kind="ExternalInput")
    eft_d = nc.dram_tensor("eft", [HIDDEN, N], f16, kind="ExternalInput")
    wqt_d = nc.dram_tensor("wqt", [HIDDEN, O], f16, kind="ExternalInput")
    wkt_d = nc.dram_tensor("wkt", [HIDDEN, O], f16, kind="ExternalInput")
    wvt_d = nc.dram_tensor("wvt", [HIDDEN, O], f16, kind="ExternalInput")
    bq_d = nc.dram_tensor("bq", [1, O], f32, kind="ExternalInput")
    bk_d = nc.dram_tensor("bk", [1, O], f32, kind="ExternalInput")
    bv_d = nc.dram_tensor("bv", [1, O], f32, kind="ExternalInput")
    lamn_d = nc.dram_tensor("lamn", [1, 1], f32, kind="ExternalInput")
    outT_d = nc.dram_tensor("outT", [O, T], f32, kind="ExternalOutput")

    with tile.TileContext(nc) as tc:
        with ExitStack() as ctx:
            const = ctx.enter_context(tc.tile_pool(name="const", bufs=1))
            proj = ctx.enter_context(tc.tile_pool(name="proj", bufs=1))
            big = ctx.enter_context(tc.tile_pool(name="big", bufs=1))

            # ---- persistent operand tiles ----
            efT = big.tile([P, IC, N], f16)
            wvT = big.tile([P, IC, O], f16)
            wkT = big.tile([P, IC, O], f16)
            xT = big.tile([P, IC, T], f16)
            wqT = big.tile([P, IC, O], f16)
            qT = proj.tile([P, OC, T], f16)          # [d-part, oc, t]
            kT = proj.tile([P, OC, N], f16)          # [d-part, oc, n]
            # [n-part, nt, h, d | 1]
            vaug = proj.tile([P, NT, HPC, HEAD + 1], f16)

            # ---- all big loads first; everything below overlaps them ----
            for ic in range(IC):
                nc.sync.dma_start(efT[:, ic, :], eft_d[ts(ic, P), :])
                nc.sync.dma_start(wvT[:, ic, :], wvt_d[ts(ic, P), :])
            for ic in range(IC):
                nc.sync.dma_start(wkT[:, ic, :], wkt_d[ts(ic, P), :])
            for ic in range(IC):
                nc.sync.dma_start(xT[:, ic, :], xt_d[ts(ic, P), :])
                nc.sync.dma_start(wqT[:, ic, :], wqt_d[ts(ic, P), :])

            # ---- biases (broadcast DMA straight from DRAM) ----
            bq_sb = const.tile([P, OC], f32)
            nc.sync.dma_start(bq_sb, bq_d[0].rearrange("(a p) -> p a", p=P))
            bk_sb = const.tile([P, OC], f32)
            nc.sync.dma_start(bk_sb, bk_d[0].rearrange("(a p) -> p a", p=P))
            bv_1 = const.tile([1, O], f32)
            nc.sync.dma_start(bv_1, bv_d[:])
            bvb = const.tile([P, O], f32)
            nc.gpsimd.partition_broadcast(bvb, bv_1)
            lam_neg = const.tile([1, 1], f32)
            nc.sync.dma_start(lam_neg, lamn_d[:])

            # ---- constants ----
            neg3 = const.tile([P, 1], f32)
            nc.vector.memset(neg3, -3.0)
            # 0/1 upper-triangular mask (keep t_local >= n_local), doubled
            # along a middle dim so one DVE mul masks both half-heads.
            tri2 = const.tile([P, 2, P], f16)
            nc.gpsimd.memset(tri2, 1.0)
            nc.gpsimd.affine_select(
                out=tri2,
                in_=tri2,
                compare_op=ALU.is_ge,
                fill=0.0,
                base=0,
                pattern=[[0, 2], [1, P]],
                channel_multiplier=-1,
            )
            ones8 = const.tile([P, HPC], f32)
            nc.vector.memset(ones8, 1.0)
            for nt_ in range(NT):
                nc.vector.tensor_copy(
                    vaug[:, nt_, :, HEAD : HEAD + 1],
                    ones8[:, :].rearrange("p (a b) -> p a b", b=1),
                )

            # ====== prefix: v projection + k(0) + q(0) ======
            with tc.tile_pool(name="ps_pjA", bufs=3, space="PSUM") as ps_pjA:
                # v[n, o] = sum_ic efT[ic].T @ WvT[ic]  (+bias)
                for nt_ in range(NT):
                    psj = ps_pjA.tile([P, 512], f32, tag="pjA", name="psv")
                    for ic in range(IC):
                        nc.tensor.matmul(
                            psj,
                            efT[:, ic, ts(nt_, P)],
                            wvT[:, ic, :],
                            start=(ic == 0),
                            stop=(ic == IC - 1),
                        )
                    nc.vector.tensor_add(
                        vaug[:, nt_, :, 0:HEAD],
                        psj[:].rearrange("p (h d) -> p h d", h=HPC),
                        bvb[:].rearrange("p (h d) -> p h d", h=HPC),
                    )
                for wT, b_sb, actT, dstT in (
                    (wkT, bk_sb, efT, kT),
                    (wqT, bq_sb, xT, qT),
                ):
                    for t2 in range(2):
                        psj = ps_pjA.tile([P, 512], f32, tag="pjA", name="psj")
                        for ic in range(IC):
                            nc.tensor.matmul(
                                psj,
                                wT[:, ic, ts(0, P)],
                                actT[:, ic, ts(t2, 512)],
                                start=(ic == 0),
                                stop=(ic == IC - 1),
                            )
                        nc.vector.tensor_scalar_add(
                            dstT[:, 0, ts(t2, 512)], psj, b_sb[:, 0:1]
                        )

            # ====== attention, software-pipelined one iteration deep ======
            with (
                tc.tile_pool(name="att_sb", bufs=4) as att_sb,
                tc.tile_pool(name="ps_qk", bufs=2, space="PSUM") as ps_qk,
                tc.tile_pool(name="ps_av", bufs=3, space="PSUM") as ps_av,
                tc.tile_pool(name="ps_pj", bufs=1, space="PSUM") as ps_pj,
            ):
                # --- q/k projection steps for oc>=1, drip-fed as PE filler ---
                proj_work = deque()  # (oc, 'mm'|'drain', fn)

                def mk_proj_steps(oc):
                    for wT, b_sb, actT, dstT in (
                        (wkT, bk_sb, efT, kT),
                        (wqT, bq_sb, xT, qT),
                    ):
                        for t2 in range(2):
                            holder = {}

                            def mk_mm(ic, wT=wT, actT=actT, t2=t2, holder=holder):
                                def f():
                                    if ic == 0:
                                        holder["ps"] = ps_pj.tile(
                                            [P, 512], f32, tag="pj", name="pj"
                                        )
                                    nc.tensor.matmul(
                                        holder["ps"],
                                        wT[:, ic, ts(oc, P)],
                                        actT[:, ic, ts(t2, 512)],
                                        start=(ic == 0),
                                        stop=(ic == IC - 1),
                                    )

                                return f

                            def mk_drain(dstT=dstT, b_sb=b_sb, t2=t2, holder=holder):
                                def f():
                                    nc.vector.tensor_scalar_add(
                                        dstT[:, oc, ts(t2, 512)],
                                        holder["ps"],
                                        b_sb[:, oc : oc + 1],
                                    )

                                return f

                            for ic in range(IC):
                                proj_work.append((oc, "mm", mk_mm(ic)))
                            proj_work.append((oc, "drain", mk_drain()))

                for oc in range(1, OC):
                    mk_proj_steps(oc)

                def pull_proj(n_mm, upto_oc=None):
                    got = 0
                    while proj_work:
                        oc_, kind, fn = proj_work[0]
                        if upto_oc is not None:
                            if oc_ > upto_oc:
                                break
                        elif got >= n_mm:
                            break
                        proj_work.popleft()
                        fn()
                        if kind == "mm":
                            got += 1

                # --- pending AV/combine work of the previous iteration ---
                pending = deque()  # ('pe'|'x', fn)

                def pull(n_pe):
                    got = 0
                    while pending:
                        kind, fn = pending[0]
                        if kind == "pe" and got >= n_pe:
                            break
                        pending.popleft()
                        fn()
                        if kind == "pe":
                            got += 1

                def queue_iter_tail(h, tcv, Es, nis):
                    avp0 = ps_av.tile([65, 512], f32, tag="av", name=f"av{h}_{tcv}0")
                    avp1 = ps_av.tile([65, 512], f32, tag="av", name=f"av{h}_{tcv}1")
                    avps = (avp0, avp1)
                    last = nis[-1]
                    for s in range(2):
                        for nt_ in nis:
                            E, w = Es[nt_]

                            def f(s=s, nt_=nt_, E=E, w=w):
                                off = 512 - w
                                nc.tensor.matmul(
                                    avps[s][:, off : off + w],
                                    vaug[:, nt_, h, :],
                                    E[:, s, :w],
                                    start=(nt_ == 0),
                                    stop=(nt_ == last),
                                )

                            pending.append(("pe", f))

                    def combine():
                        # row 64 (sum E_s) out of PSUM via a start-0 DVE
                        # copy, SBUF-to-SBUF DMA down to partition 0, fast
                        # reciprocal (the neg half scaled by host-computed
                        # -lambda), gpsimd partition broadcast.
                        Pc = att_sb.tile(
                            [128, 2, 512], f32, tag="Pc", bufs=2, name="Pc"
                        )
                        nc.vector.tensor_copy(Pc[0:65, 0, :], avp0[0:65, :])
                        nc.vector.tensor_copy(Pc[0:65, 1, :], avp1[0:65, :])
                        Sh = att_sb.tile([1, 2, 512], f32, tag="Sh", bufs=2, name="Sh")
                        nc.sync.dma_start(Sh[:, 0, :], Pc[64:65, 0, :])
                        nc.sync.dma_start(Sh[:, 1, :], Pc[64:65, 1, :])
                        Rh = att_sb.tile([1, 2, 512], f32, tag="Rh", bufs=2, name="Rh")
                        nc.vector.reciprocal_approx_fast(out=Rh, in_=Sh)
                        nc.vector.tensor_scalar_mul(Rh[:, 1, :], Rh[:, 1, :], lam_neg)
                        Rb = att_sb.tile([64, 2, 512], f32, tag="Rb", bufs=2, name="Rb")
                        nc.gpsimd.partition_broadcast(Rb, Rh)
                        m = att_sb.tile([64, 2, 512], f32, tag="m", bufs=2, name="m")
                        nc.vector.tensor_mul(m[:, 0, :], avp0[0:64, :], Rb[:, 0, :])
                        nc.vector.tensor_mul(m[:, 1, :], avp1[0:64, :], Rb[:, 1, :])
                        o = att_sb.tile([64, 512], f32, tag="o", bufs=3, name="o")
                        nc.vector.tensor_add(o, m[:, 0, :], m[:, 1, :])
                        nc.sync.dma_start(
                            outT_d[64 * h : 64 * h + 64, ts(tcv, 512)], o
                        )

                    pending.append(("x", combine))

                # --- the pipelined loop ---
                iters = [(h, tcv) for h in range(HPC) for tcv in (1, 0)]
                for h, tcv in iters:
                    oc, j = h // 2, h % 2
                    if j == 0 and tcv == 1 and oc >= 1:
                        # round boundary: this head needs qT/kT[:, oc, :]
                        pull_proj(0, upto_oc=oc)
                    nis = list(range(4)) if tcv == 0 else list(range(NT))
                    Es = {}
                    for idx, nt_ in enumerate(nis):
                        t0 = nt_ * P
                        cs = max(t0, 512 * tcv)
                        w = 512 * (tcv + 1) - cs
                        att_ps = ps_qk.tile(
                            [P, 2, 512], f32, tag="qk", name="attps"
                        )
                        E = att_sb.tile(
                            [P, 2, 512], f16, tag="E", bufs=18, name="E"
                        )
                        Es[nt_] = (E, w)
                        for s in range(2):
                            base = 64 * j + 32 * s
                            nc.tensor.matmul(
                                att_ps[:, s, :w],
                                kT[base : base + 32, oc, ts(nt_, P)],
                                qT[base : base + 32, oc, cs : cs + w],
                                start=True,
                                stop=True,
                                tile_position=(96, 0) if base == 96 else None,
                            )
                        # bias shifts all exps by e^-3 (cancels in P/s),
                        # keeping E inside fp16 range
                        nc.scalar.activation(
                            E[:, :, :w],
                            att_ps[:, :, :w],
                            AF.Exp,
                            bias=neg3[:, 0:1],
                            scale=SCALE,
                        )
                        if cs == t0:
                            # diagonal block: keep t_local >= n_local
                            nc.vector.tensor_mul(E[:, :, 0:P], E[:, :, 0:P], tri2)
                        pull(2 if tcv == 1 else 4)
                        pull_proj(1 if idx % 2 == 0 else 2)
                    queue_iter_tail(h, tcv, Es, nis)
                # drain everything left
                pull(10**9)
                pull_proj(10**9, upto_oc=OC)

    nc.compile()
    return nc


def _ensure_axon_hooks():
    """concourse's trace path imports antenv.axon_hooks, which this image
    lacks; provide it (registering the real ctypes NTFF hook when available)
    so BASS_TRACE=1 degrades gracefully instead of crashing."""
    import sys
    import types

    if "antenv.axon_hooks" in sys.modules:
        return
    try:
        import antenv.axon_hooks  # noqa: F401

        return
    except ImportError:
        pass
    mod = types.ModuleType("antenv.axon_hooks")
    mod._hook = None
    mod.set_axon_ntff_profile_hook = lambda h: setattr(mod, "_hook", h)
    mod.get_axon_ntff_profile_hook = lambda: mod._hook
    sys.modules["antenv.axon_hooks"] = mod
    import os

    if os.environ.get("KERNEL_TRACE") == "1":
        try:
            from trn_agent_boot.trn_boot import _ntff_profile_via_ctypes

            mod._hook = _ntff_profile_via_ctypes("/opt/axon/libaxon_pjrt.so")
        except Exception:
            pass


def _get_state():
    if "nc" not in _STATE:
        from concourse.bass_utils import run_bass_kernel_spmd

        _ensure_axon_hooks()
        _STATE["nc"] = _build_nc()
        _STATE["run"] = run_bass_kernel_spmd
    return _STATE


def kernel(**inputs):
    st = _get_state()

    def f32c(a):
        return np.ascontiguousarray(np.asarray(a, dtype=np.float32))

    x = np.asarray(inputs["x"], dtype=np.float32)
    ef = np.asarray(inputs["encoder_feature"], dtype=np.float32)
    Wq, bq = np.asarray(inputs["Wq"], np.float32), np.asarray(inputs["bq"], np.float32)
    Wk, bk = np.asarray(inputs["Wk"], np.float32), np.asarray(inputs["bk"], np.float32)
    Wv, bv = np.asarray(inputs["Wv"], np.float32), np.asarray(inputs["bv"], np.float32)
    lq1 = f32c(inputs["lambda_q1"]).ravel()
    lq2 = f32c(inputs["lambda_q2"]).ravel()
    lk1 = f32c(inputs["lambda_k1"]).ravel()
    lk2 = f32c(inputs["lambda_k2"]).ravel()

    lam1 = np.exp(np.sum(lq1 * lk1, dtype=np.float32))
    lam2 = np.exp(np.sum(lq2 * lk2, dtype=np.float32))
    lam_full = np.float32(lam1 - lam2 + LAMBDA_INIT)
    lamn = np.array([[-lam_full]], dtype=np.float32)

    in_maps = []
    for c in range(NCORES):
        b, hg = c // 2, c % 2
        sl = slice(hg * O, (hg + 1) * O)
        in_maps.append(
            {
                "xt": np.ascontiguousarray(x[b].T.astype(np.float16)),
                "eft": np.ascontiguousarray(ef[b].T.astype(np.float16)),
                "wqt": np.ascontiguousarray(Wq[sl].T.astype(np.float16)),
                "wkt": np.ascontiguousarray(Wk[sl].T.astype(np.float16)),
                "wvt": np.ascontiguousarray(Wv[sl].T.astype(np.float16)),
                "bq": f32c(bq[sl]).reshape(1, O),
                "bk": f32c(bk[sl]).reshape(1, O),
                "bv": f32c(bv[sl]).reshape(1, O),
                "lamn": lamn,
            }
        )

    res = st["run"](st["nc"], in_maps, core_ids=list(range(NCORES)))
    _STATE["last_results"] = res

    out = np.empty((B, T, HIDDEN), dtype=np.float32)
    for c in range(NCORES):
        b, hg = c // 2, c % 2
        out[b, :, hg * O : (hg + 1) * O] = res.results[c]["outT"].T
    return out
